# revision 34
# baseline (speedup 1.0000x reference)
"""Trainium2 Bass kernel for BlockUncertaintyTracker (segment_reduce).

Per 4x4 block of a [16,1,2048,2048] batch: mean and 0.9-quantile
(= 0.5*(2nd+3rd largest of 16)) averaged over batch, EMA update, then the
ratio broadcast back to full shape.

Sharding: spatial over H across 8 cores (256 image rows / 64 block rows per
core); every core sees all 16 batches for its rows, no collectives.

Design notes:
  - Loads are SWDGE cast-DMAs (f32 HBM -> f16 SBUF): no ScalarE cast pass,
    half the SBUF traffic. Group 0 is split (256/1792 cols) so the vector
    engine starts as soon as the first 512 KiB lands.
  - Mean path: contiguous-rhs f16 matmuls accumulate a [128, 2048] PSUM
    tile; one 4->1 column fold per w-half at the end (strided-rhs f32
    matmuls in the original were ~5x slower and dominated TensorE).
  - Quantile: vertical top-3 via fused-wide f16 CEs on DVE (2x mode),
    even/odd deinterleaves on ScalarE, 9-op sorted merges on DVE; c2/c3 are
    accumulated by two matmuls so no DVE op is spent on their sum.
  - Group 7 + the tail (EMA update, ratio, x4 expansion) + the writes are
    split into W-halves: the left half's 8 MiB of writes overlap the right
    half's compute.
  - Output map is stored f16 on device (quantization ~5e-4 rel, vs the 2e-2
    gate) and widened to f32 on the host during the unshard/gather step.
  - PSUM matmul accumulation: a start=True matmul zeroes the WHOLE 2KB bank
    it touches. Matmul chunks are clipped to absolute 512-f32 bank
    boundaries and only the chronologically-first matmul per bank carries
    start=True (later fresh regions rely on has_written overwrite).
  - No finer W-chunking: per-op DVE overhead (~60cyc + dispatch) of chunked
    variants costs more than the write overlap it buys at 16 MiB of writes.
"""

import os

import numpy as np

# ---- problem constants (hardcoded; kernel.py must be self-contained) ----
B = 16          # batch
H = 2048
W = 2048
BS = 4          # block size
NCORES = 8
HS = H // NCORES            # 256 rows per core
NBH = HS // BS              # 64 block rows per core
NBW = W // BS               # 512 block cols
ROWS = B * HS               # 4096 rows in a per-core slab
NG = 8                      # groups per core; each = 2 batches x 256 rows
GB = B // NG                # 2 batches per group
DECAY = 0.99
ALPHA = 0.1
EPS = 1e-5
C_MEAN = (1.0 - DECAY) / (BS * BS * B)    # fold mean-over-16-elems and batch
C_QUANT = (1.0 - DECAY) * 0.5 / B         # fold 0.5*(m2+m3) and batch mean

_CACHE = {}


def _split_multi_waits(nc):
    """This walrus build encodes at most ONE sync wait per instruction.
    Tile attaches several. Hoist excess waits onto same-engine NOPs placed
    immediately before the owning instruction."""
    import concourse.mybir as mybir

    plans = []
    for f in nc.m.functions:
        for bb in f.blocks:
            for inst in bb.instructions:
                si = getattr(inst, "sync_info", None)
                waits = list(si.on_wait) if (si and si.on_wait) else []
                if len(waits) > 1:
                    si.on_wait = [waits[-1]]
                    plans.append((inst.name, waits[:-1]))

    if not plans:
        return

    nop_for = {}
    stray = set()
    for iname, extra in plans:
        nops = []
        for w in extra:
            nop = nc.engines[nc.inst_map[iname].engine].nop(nofuse=True).ins
            nop.sync_info = mybir.SyncInfo(on_wait=[w], on_update=[])
            nops.append(nop)
            stray.add(nop.name)
        nop_for[iname] = nops

    for f in nc.m.functions:
        for bb in f.blocks:
            out = []
            changed = False
            for inst in bb.instructions:
                if inst.name in stray:
                    changed = True
                    continue
                if inst.name in nop_for:
                    out.extend(nop_for[inst.name])
                    changed = True
                out.append(inst)
            if changed:
                bb.instructions = out


def _build():
    """Builds the single-core Bass program (SPMD across 8 cores)."""
    from contextlib import ExitStack

    import concourse.bass as bass
    import concourse.mybir as mybir
    import concourse.tile as tile

    f32 = mybir.dt.float32
    f16 = mybir.dt.float16
    MAX = mybir.AluOpType.max
    MIN = mybir.AluOpType.min
    MULT = mybir.AluOpType.mult
    ADD = mybir.AluOpType.add
    COPY = mybir.ActivationFunctionType.Copy

    nc = bass.Bass("TRN2", target_bir_lowering=False, debug=False)

    x = nc.dram_tensor("x", [ROWS, W], f32, kind="ExternalInput").ap()
    ee = nc.dram_tensor("ee", [NBH, NBW], f32, kind="ExternalInput").ap()
    eq = nc.dram_tensor("eq", [NBH, NBW], f32, kind="ExternalInput").ap()
    # ones[p, m] = (p % 64 == m // 2): batch-pair fold + row duplication
    ones = nc.dram_tensor("ones", [128, 128], f32, kind="ExternalInput").ap()
    y = nc.dram_tensor("y", [ROWS, W], f16, kind="ExternalOutput").ap()

    # input rows = ((g*2+b2)*64+i)*4 + r; per g: [128=(b2 i), r, w]
    xr = x.rearrange("(g b2 i r) w -> g (b2 i) r w", g=NG, b2=GB, i=NBH, r=BS)
    # output rows = b*256 + 4i + 2h + r2; per (b, h, w-half v): [(i r2), w]
    yr = y.rearrange(
        "(b i h r2) (v w) -> b h v i r2 w", b=B, i=NBH, h=2, r2=2, v=2
    )

    with tile.TileContext(nc) as tc, ExitStack() as ctx:
        pool = ctx.enter_context(tc.tile_pool(name="work", bufs=1))
        ppool = ctx.enter_context(tc.tile_pool(name="acc", bufs=1, space="PSUM"))

        ones_sb = pool.tile([128, 128], f32, tag="ones")
        nc.sync.dma_start(ones_sb[:, :], ones)
        ones16 = pool.tile([128, 128], f16, tag="ones16")
        nc.scalar.copy(ones16[:, :], ones_sb[:, :])

        ee_sb = pool.tile([128, NBW], f32, tag="ema", bufs=4, name="ee_sb")
        nc.sync.dma_start(
            ee_sb[:, :], ee.unsqueeze(1).broadcast_to((NBH, 2, NBW))
        )
        eq_sb = pool.tile([128, NBW], f32, tag="ema", bufs=4, name="eq_sb")
        nc.sync.dma_start(
            eq_sb[:, :], eq.unsqueeze(1).broadcast_to((NBH, 2, NBW))
        )
        # EMA affines on ScalarE (early; only depend on the small loads)
        ee2 = pool.tile([128, NBW], f32, tag="ema", bufs=4, name="ee2")
        nc.scalar.activation(ee2[:, :], ee_sb[:, :], COPY, bias=EPS, scale=DECAY)
        eq2 = pool.tile([128, NBW], f32, tag="ema", bufs=4, name="eq2")
        nc.scalar.activation(eq2[:, :], eq_sb[:, :], COPY, bias=0.0, scale=DECAY)

        psum_s = ppool.tile([128, W], f32, tag="ps")
        psum_q = ppool.tile([128, NBW], f32, tag="pq")

        def tt(dst, a, bb, op):
            nc.vector.tensor_tensor(dst, a, bb, op)

        def deint(src, wout, tag, bufs, nm):
            v = src.rearrange("p (j two) -> p j two", two=2)
            te = pool.tile([128, wout], f16, tag=tag, bufs=bufs, name=nm + "e")
            nc.scalar.copy(te[:, :], v[:, :, 0])
            to = pool.tile([128, wout], f16, tag=tag, bufs=bufs, name=nm + "o")
            nc.scalar.copy(to[:, :], v[:, :, 1])
            return te, to

        def tail_and_writes(v):
            """EMA update + ratio + writes for w-half v (cols 1024v..1024v+1024).
            Emitted right after the last group's matching half so the first
            half's writes overlap the second half's compute."""
            hw = W // 2                  # 1024 cols
            j0 = v * (hw // BS)          # 256-block offset
            jn = hw // BS
            ssb = pool.tile([128, hw], f32, tag=f"ssb{v}", name=f"ssb{v}")
            # ScalarE does the PSUM->SBUF move: it's idle here and this
            # keeps the fold chain off the DVE critical path
            nc.scalar.copy(ssb[:, :], psum_s[:, v * hw : (v + 1) * hw])
            sv = ssb.rearrange("p (j two) -> p j two", two=2)
            a2 = pool.tile([128, hw // 2], f32, tag=f"tl_a2{v}", name=f"a2{v}")
            tt(a2[:, :], sv[:, :, 0], sv[:, :, 1], ADD)
            av = a2.rearrange("p (j two) -> p j two", two=2)
            s4 = pool.tile([128, jn], f32, tag=f"tl_s4{v}", name=f"s4{v}")
            tt(s4[:, :], av[:, :, 0], av[:, :, 1], ADD)

            den = pool.tile([128, jn], f32, tag=f"tl_den{v}", name=f"den{v}")
            nc.vector.scalar_tensor_tensor(
                den[:, :], s4[:, :], C_MEAN, ee2[:, j0 : j0 + jn],
                op0=MULT, op1=ADD,
            )
            num = pool.tile([128, jn], f32, tag=f"tl_num{v}", name=f"num{v}")
            nc.vector.scalar_tensor_tensor(
                num[:, :], psum_q[:, j0 : j0 + jn], C_QUANT,
                eq2[:, j0 : j0 + jn], op0=MULT, op1=ADD,
            )
            rec = pool.tile([128, jn], f32, tag=f"tl_rec{v}", name=f"rec{v}")
            nc.vector.reciprocal(rec[:, :], den[:, :])
            u = pool.tile([128, jn], f32, tag=f"tl_u{v}", name=f"u{v}")
            tt(u[:, :], num[:, :], rec[:, :], MULT)

            # expand x4 along columns, casting to f16, on ScalarE: one op
            # with a stride-0 broadcast READ and contiguous writes (strided
            # sub-32-bit ACT writes pay a ~2x read-modify-write penalty)
            u16 = pool.tile([128, hw], f16, tag=f"u16_{v}", name=f"u16_{v}")
            u16v = u16.rearrange("p (j c) -> p j c", c=BS)
            ub = u[:, :].unsqueeze(2).broadcast_to((128, jn, BS))
            nc.scalar.copy(u16v, ub)

            # 32 writes for this half, alternated across both HWDGE
            # rings (SP and ACT) to parallelize descriptor issue
            for b in range(B):
                for h in range(2):
                    eng = nc.sync if (b % 2 == 0) else nc.scalar
                    eng.dma_start(yr[b, h, v], u16[:, :])

        # (g, w-slice) work items; g0 split to shorten the ramp, g7 split so
        # the left half's tail+writes overlap the right half's compute
        # item boundaries MUST be 512-col (PSUM bank) aligned: a start=True
        # matmul zeroes the whole bank it touches
        items = (
            [(0, 0, 256), (0, 256, W)]
            + [(g, 0, W) for g in range(1, NG - 1)]
            + [(NG - 1, 0, W // 2), (NG - 1, W // 2, W)]
        )
        # a start=True matmul zeroes the whole 2KB PSUM bank it touches, so
        # only the chronologically-first matmul per bank may carry it; later
        # first-writes of fresh regions rely on has_written overwrite
        started_banks = set()
        n_items = len(items)

        for it, (g, ws, we) in enumerate(items):
            cw = we - ws                 # column width of this item
            hk = cw // 2
            jk = cw // BS
            sfx = f"{g}_{ws}"
            first = it == 0
            last = it == n_items - 1

            # ---- load: SWDGE cast f32 -> f16, [128, (r, w)] ----
            gt = pool.tile([128, BS * cw], f16, tag="gt", bufs=3, name=f"gt_{sfx}")
            gtv = gt.rearrange("p (r w) -> p r w", r=BS)
            nc.gpsimd.dma_start(gtv, xr[g][:, :, ws:we])

            # ---- mean path: contiguous-rhs matmuls into psum_s ----
            # chunks are clipped to absolute 512-col (PSUM bank) boundaries
            for r in range(BS):
                a = ws
                while a < we:
                    a1 = min(we, (a // 512 + 1) * 512)
                    bank_new = a // 512 not in started_banks
                    nc.tensor.matmul(
                        psum_s[:, a:a1],
                        lhsT=ones16[:, :],
                        rhs=gt[:, r * cw + (a - ws) : r * cw + (a1 - ws)],
                        start=bank_new,
                        stop=(g == NG - 1 and r == BS - 1),
                    )
                    started_banks.add(a // 512)
                    a = a1

            # ---- vertical: sorted top-3 of the 4 rows, fused CEs ----
            big1 = pool.tile([128, 2 * cw], f16, tag="big", bufs=3,
                             name=f"big1_{sfx}")
            tt(big1[:, :], gt[:, 0 : 2 * cw], gt[:, 2 * cw : 4 * cw], MAX)
            big2 = pool.tile([128, 2 * cw], f16, tag="big", bufs=3,
                             name=f"big2_{sfx}")
            tt(big2[:, :], gt[:, 0 : 2 * cw], gt[:, 2 * cw : 4 * cw], MIN)
            m = pool.tile([128, cw], f16, tag="vp", bufs=6, name=f"m_{sfx}")
            tt(m[:, :], big1[:, 0:cw], big1[:, cw : 2 * cw], MAX)
            t1 = pool.tile([128, cw], f16, tag="vp", bufs=6, name=f"t1_{sfx}")
            tt(t1[:, :], big1[:, 0:cw], big1[:, cw : 2 * cw], MIN)
            t2 = pool.tile([128, cw], f16, tag="vp", bufs=6, name=f"t2_{sfx}")
            tt(t2[:, :], big2[:, 0:cw], big2[:, cw : 2 * cw], MAX)
            s2 = pool.tile([128, cw], f16, tag="vp", bufs=6, name=f"s2_{sfx}")
            tt(s2[:, :], t1[:, :], t2[:, :], MAX)
            t3 = pool.tile([128, cw], f16, tag="vp", bufs=6, name=f"t3_{sfx}")
            tt(t3[:, :], t1[:, :], t2[:, :], MIN)

            # ---- a-level: merge sorted triples of adjacent columns ----
            me, mo = deint(m, hk, "ad", 8, f"m_{sfx}")
            s2e, s2o = deint(s2, hk, "ad", 8, f"s2_{sfx}")
            t3e, t3o = deint(t3, hk, "ad", 8, f"t3_{sfx}")
            p1 = pool.tile([128, hk], f16, tag="al", bufs=8, name=f"p1_{sfx}")
            tt(p1[:, :], me[:, :], mo[:, :], MAX)
            u1 = pool.tile([128, hk], f16, tag="al", bufs=8, name=f"u1_{sfx}")
            tt(u1[:, :], me[:, :], mo[:, :], MIN)
            u2 = pool.tile([128, hk], f16, tag="al", bufs=8, name=f"u2_{sfx}")
            tt(u2[:, :], s2e[:, :], s2o[:, :], MAX)
            p2 = pool.tile([128, hk], f16, tag="al", bufs=8, name=f"p2_{sfx}")
            tt(p2[:, :], u1[:, :], u2[:, :], MAX)
            w2 = pool.tile([128, hk], f16, tag="al", bufs=8, name=f"w2_{sfx}")
            tt(w2[:, :], me[:, :], s2o[:, :], MIN)
            w3 = pool.tile([128, hk], f16, tag="al", bufs=8, name=f"w3_{sfx}")
            tt(w3[:, :], s2e[:, :], mo[:, :], MIN)
            w4 = pool.tile([128, hk], f16, tag="al", bufs=8, name=f"w4_{sfx}")
            tt(w4[:, :], w2[:, :], w3[:, :], MAX)
            w1 = pool.tile([128, hk], f16, tag="al", bufs=8, name=f"w1_{sfx}")
            tt(w1[:, :], t3e[:, :], t3o[:, :], MAX)
            p3 = pool.tile([128, hk], f16, tag="al", bufs=8, name=f"p3_{sfx}")
            tt(p3[:, :], w1[:, :], w4[:, :], MAX)

            # ---- b-level: c2 = 2nd, c3 = 3rd of 16 per block ----
            p1e, p1o = deint(p1, jk, "bd", 8, f"p1_{sfx}")
            p2e, p2o = deint(p2, jk, "bd", 8, f"p2_{sfx}")
            p3e, p3o = deint(p3, jk, "bd", 8, f"p3_{sfx}")
            z1 = pool.tile([128, jk], f16, tag="bl", bufs=8, name=f"z1_{sfx}")
            tt(z1[:, :], p1e[:, :], p1o[:, :], MIN)
            z2 = pool.tile([128, jk], f16, tag="bl", bufs=8, name=f"z2_{sfx}")
            tt(z2[:, :], p2e[:, :], p2o[:, :], MAX)
            c2 = pool.tile([128, jk], f16, tag="bl", bufs=8, name=f"c2_{sfx}")
            tt(c2[:, :], z1[:, :], z2[:, :], MAX)
            z4 = pool.tile([128, jk], f16, tag="bl", bufs=8, name=f"z4_{sfx}")
            tt(z4[:, :], p1e[:, :], p2o[:, :], MIN)
            z5 = pool.tile([128, jk], f16, tag="bl", bufs=8, name=f"z5_{sfx}")
            tt(z5[:, :], p2e[:, :], p1o[:, :], MIN)
            z6 = pool.tile([128, jk], f16, tag="bl", bufs=8, name=f"z6_{sfx}")
            tt(z6[:, :], z4[:, :], z5[:, :], MAX)
            z3 = pool.tile([128, jk], f16, tag="bl", bufs=8, name=f"z3_{sfx}")
            tt(z3[:, :], p3e[:, :], p3o[:, :], MAX)
            c3 = pool.tile([128, jk], f16, tag="bl", bufs=8, name=f"c3_{sfx}")
            tt(c3[:, :], z3[:, :], z6[:, :], MAX)
            # qs = c2 + c3 folded into the PSUM accumulation (2 matmuls)
            jb = ws // BS
            # psum_q is a single 2KB bank: a second start=True in the same
            # bank zeroes the whole bank, so only the very first matmul
            # may carry it (item 2's region is fresh-written via has_written)
            nc.tensor.matmul(
                psum_q[:, jb : jb + jk], lhsT=ones16[:, :], rhs=c2[:, :],
                start=(it == 0), stop=False,
            )
            nc.tensor.matmul(
                psum_q[:, jb : jb + jk], lhsT=ones16[:, :], rhs=c3[:, :],
                start=False, stop=(g == NG - 1),
            )

            if g == NG - 1:
                tail_and_writes(0 if we == W // 2 else 1)

    _split_multi_waits(nc)
    return nc


def _get_nc():
    if "nc" not in _CACHE:
        _CACHE["nc"] = _build()
    return _CACHE["nc"]


def kernel(current_errors, ema_errors, ema_quantile):
    from concourse.bass_utils import run_bass_kernel_spmd

    x = np.asarray(current_errors, dtype=np.float32).reshape(B, H, W)
    ee = np.asarray(ema_errors, dtype=np.float32).reshape(H // BS, W // BS)
    eq = np.asarray(ema_quantile, dtype=np.float32).reshape(H // BS, W // BS)

    # ones[p, m] == 1 iff p % 64 == m // 2
    ones = np.zeros((128, 128), dtype=np.float32)
    p = np.arange(128)
    ones[p, (p % NBH) * 2] = 1.0
    ones[p, (p % NBH) * 2 + 1] = 1.0

    in_maps = []
    for k in range(NCORES):
        xs = np.ascontiguousarray(x[:, k * HS : (k + 1) * HS, :]).reshape(ROWS, W)
        ees = np.ascontiguousarray(ee[k * NBH : (k + 1) * NBH, :])
        eqs = np.ascontiguousarray(eq[k * NBH : (k + 1) * NBH, :])
        in_maps.append({"x": xs, "ee": ees, "eq": eqs, "ones": ones})

    nc = _get_nc()
    trace = bool(int(os.environ.get("KERNEL_TRACE", "0")))
    try:
        res = run_bass_kernel_spmd(
            nc, in_maps, core_ids=list(range(NCORES)), trace=trace
        )
    except Exception:
        # transient device state (e.g. NRT_EXEC_UNIT_UNRECOVERABLE) — retry once
        res = run_bass_kernel_spmd(
            nc, in_maps, core_ids=list(range(NCORES)), trace=trace
        )
    _CACHE["last_results"] = res

    out = np.empty((B, 1, H, W), dtype=np.float32)
    for k in range(NCORES):
        out[:, 0, k * HS : (k + 1) * HS, :] = (
            res.results[k]["y"].astype(np.float32).reshape(B, HS, W)
        )
    return out
